# revision 39
# baseline (speedup 1.0000x reference)
"""Multi-head attention (B=4, S=2048, D=1024, H=16) on 8 TRN2 NeuronCores.

Sharding: token-parallel, zero-collective. Core c handles batch c//2 and
half (c%2) of that batch's sequence as queries (1024 tokens); it computes
K/V for the batch's full 2048 tokens locally (work duplicated x2 across the
pair, but no cross-core communication at all).

The host reorders tokens per-core so every core's own query tokens are
always columns [0, 1024) of its xT input -> all 8 cores run the identical
SPMD graph. Key order within a batch differs per core, but attention is
permutation-invariant over keys.

On-device layout is "transposed" (channels x tokens) so no transposes:
  Q^T = WqT.T @ xT      (bias via K=1 ones-row matmul into PSUM)
  S^T = K^T_head.T @ Q^T_head   (contraction over head_dim=64)
  E   = exp(S^T / 8)    (ScalarE, reads PSUM, 2 key-tiles per ACTIVATE)
  PV  = V'_head.T @ E   (V' has a ones column -> row 64 = softmax denom)
  out = attnT.T @ WoT   (natural token-major output, contiguous DMA)

The attention phase is choreographed for the PE clock-gate (HAM): the PE
must never stall on a semaphore or it drops to 1.2 GHz. Scores(i) ->
exp(i) -> pv(i-2) software pipelining plus real filler work (the deferred
qb1 Q-projection during qb0, and qb0's output projection during qb1) keep
the TensorE queue saturated while ScalarE streams exps back-to-back; an
explicit add_dep chain pins the PE order so the tile scheduler cannot
re-serialize it.

Precision: projections + output in fp32r (fp32 with 11-bit mantissa, full
TensorE rate, host pre-rounds inputs); attention QK/PV in bf16;
accumulation always fp32 in PSUM. Softmax denominators via DVE
reciprocal_approx_fast + gpsimd partition_broadcast.
"""

import numpy as np

D = 1024
H = 16
HD = 64
B = 4
S = 2048
TQ = 1024  # query tokens per core
N_CORES = 8

_CACHE: dict = {}


def _round_fp32r(x: np.ndarray) -> np.ndarray:
    """Round-to-nearest-even fp32 mantissa to 11 bits (fp32r)."""
    b = np.ascontiguousarray(x, dtype=np.float32).view(np.uint32).astype(np.uint64)
    out = ((b + 0x7FF + ((b >> 12) & 1)) & 0xFFFFF000).astype(np.uint32)
    return out.view(np.float32).reshape(x.shape)


def _build_nc():
    import concourse.tile as tile
    import concourse.mybir as mybir
    from concourse import bacc
    from concourse.bass import ds
    from concourse.tile import add_dep_helper

    f32 = mybir.dt.float32
    f32r = mybir.dt.float32r
    bf16 = mybir.dt.bfloat16
    EXP = mybir.ActivationFunctionType.Exp

    nc = bacc.Bacc()

    XTQ = nc.declare_dram_parameter("xTq", [D, TQ], f32r, isOutput=False)
    XTO = nc.declare_dram_parameter("xTo", [D, TQ], f32r, isOutput=False)
    WQT = nc.declare_dram_parameter("wqT", [D, D], f32r, isOutput=False)
    WKT = nc.declare_dram_parameter("wkT", [D, D], f32r, isOutput=False)
    WVT = nc.declare_dram_parameter("wvT", [D, D], f32r, isOutput=False)
    WOT = nc.declare_dram_parameter("woT", [D, D], f32r, isOutput=False)
    BIAS = nc.declare_dram_parameter("biases", [1, 2 * D], f32r, isOutput=False)
    ONESR = nc.declare_dram_parameter("onesrow", [1, 512], f32r, isOutput=False)
    BCOLS = nc.declare_dram_parameter("biascols", [128, 16], f32, isOutput=False)
    ONESC = nc.declare_dram_parameter("onescol", [128, 1], bf16, isOutput=False)
    OUT = nc.declare_dram_parameter("out", [TQ, D], f32, isOutput=True)

    KT = S // 128   # 16 key tiles
    DT = D // 128   # 8 channel tiles
    GPH = KT // 2   # 8 score groups (2 key-tiles each) per head

    pe_chain = [None]

    def chain(bi):
        if pe_chain[0] is not None:
            add_dep_helper(
                bi.ins, pe_chain[0].ins, sync=False, reason="PE order chain"
            )
        pe_chain[0] = bi
        return bi

    with tile.TileContext(nc) as tc:
        with (
            tc.tile_pool(name="pers", bufs=1) as pers,
            tc.tile_pool(name="epool", bufs=4) as epool,
            tc.tile_pool(name="xtq", bufs=1) as xtq,
            tc.tile_pool(name="wsl", bufs=3) as wsl,
        ):
            # ---- persistent tiles --------------------------------------
            kT = pers.tile([128, DT, S], bf16)          # K^T  (ch x tok)
            qT = pers.tile([128, DT, TQ], bf16)         # Q^T  (ch x tok)
            vP = pers.tile([128, KT, H, HD + 1], bf16)  # V'  (+ones col)
            biases = pers.tile([1, 2 * D], f32r)
            onesrow = pers.tile([1, 512], f32r)
            onescol = pers.tile([128, 1], bf16)
            bcols = pers.tile([128, 16], f32)
            scratch = pers.tile([1, 16], f32)

            # setup + bulk prefetches ride the ScalarE HWDGE queue so the
            # sync queue carries only the critical path (xTq h1, wq slices)
            nc.scalar.dma_start(bcols[:], BCOLS[:])
            nc.scalar.dma_start(biases[:], BIAS[:])
            nc.scalar.dma_start(onesrow[:], ONESR[:])
            nc.scalar.dma_start(onescol[:], ONESC[:])

            # warm the exp table early (one-time ~2.7us ACT table load)
            nc.vector.memset(scratch[:], 0.0)
            nc.scalar.activation(scratch[0:1, 0:16], scratch[0:1, 0:16], EXP)

            # V' ones columns (softmax denominator trick)
            for t in range(KT):
                nc.vector.tensor_copy(
                    vP[:, t, :, HD], onescol[:, 0:1].broadcast_to([128, H])
                )

            xTq = xtq.tile([128, DT, TQ], f32r)  # own-token half of x^T
            nc.sync.dma_start(
                xTq[:, ds(0, 4), :],
                XTQ[ds(0, 512), :].rearrange("(t p) s -> p t s", p=128),
            )
            nc.scalar.dma_start(
                xTq[:, ds(4, 4), :],
                XTQ[ds(512, 512), :].rearrange("(t p) s -> p t s", p=128),
            )

            def x_lhsT(t, k):
                # token-tile t (128 tokens), channel-tile k -> [128, 128]
                if t < 8:
                    return xTq[:, k, ds(t * 128, 128)]
                return xTo[:, k, ds((t - 8) * 128, 128)]

            def x_rhs(tb, k):
                # 512-token block tb, channel-tile k -> [128, 512]
                if tb < 2:
                    return xTq[:, k, ds(tb * 512, 512)]
                return xTo[:, k, ds((tb - 2) * 512, 512)]

            def proj_chain(ps, lhsT_fn, rhs_fn, bias_lhsT=None, bias_rhs=None):
                has_bias = bias_lhsT is not None
                for k in range(DT):
                    chain(nc.tensor.matmul(
                        ps[:], lhsT_fn(k), rhs_fn(k),
                        start=(k == 0), stop=(not has_bias and k == DT - 1),
                    ))
                if has_bias:
                    chain(nc.tensor.matmul(
                        ps[:], bias_lhsT, bias_rhs, start=False, stop=True,
                    ))

            def q_chain(psQ, m, tb):
                wq = wsl.tile([128, DT, 128], f32r, tag="w")
                nc.sync.dma_start(
                    wq[:],
                    WQT[:, ds(m * 128, 128)].rearrange("(t p) m -> p t m", p=128),
                )
                ps = psQ.tile([128, 512], f32, tag="pq")
                for k in range(DT):
                    chain(nc.tensor.matmul(
                        ps[:], wq[:, k, :], x_rhs(tb, k),
                        start=(k == 0), stop=(k == DT - 1),
                    ))
                nc.vector.tensor_scalar(
                    out=qT[:, m, ds(tb * 512, 512)],
                    in0=ps[:],
                    scalar1=bcols[:, m : m + 1],
                    scalar2=None,
                    op0=mybir.AluOpType.add,
                )

            with tc.tile_pool(name="psA", bufs=2, space="PSUM") as psA:
                with tc.tile_pool(name="xto", bufs=1) as xto:
                    with tc.tile_pool(name="wv", bufs=2) as wvp:
                        # prefetch: wv chunk 0 + other-half x^T land while
                        # the PE runs the Q-projection below
                        wv_tiles = []
                        wv_tiles.append(wvp.tile([128, DT, 512], f32r, tag="wv", name="wv0"))
                        nc.scalar.dma_start(
                            wv_tiles[0][:],
                            WVT[:, ds(0, 512)].rearrange("(t p) m -> p t m", p=128),
                        )
                        xTo = xto.tile([128, DT, TQ], f32r)  # other-token half
                        nc.scalar.dma_start(
                            xTo[:], XTO[:].rearrange("(t p) s -> p t s", p=128)
                        )
                        wv_tiles.append(wvp.tile([128, DT, 512], f32r, tag="wv", name="wv1"))
                        nc.scalar.dma_start(
                            wv_tiles[1][:],
                            WVT[:, ds(512, 512)].rearrange("(t p) m -> p t m", p=128),
                        )

                        # ---- Q^T projection qb0 first: only needs xTq ----
                        for m in range(DT):
                            q_chain(psA, m, 0)

                        # ---- V projection (wvT in two halves) ------------
                        for c in range(2):
                            wv = wv_tiles[c]
                            for t in range(KT):
                                ps = psA.tile([128, 512], f32, tag="pj")
                                proj_chain(
                                    ps,
                                    lambda k, t=t: x_lhsT(t, k),
                                    lambda k, wv=wv: wv[:, k, :],
                                    onesrow[0:1, 0:128],
                                    biases[0:1, ds(c * 512, 512)],
                                )
                                nc.vector.tensor_copy(
                                    vP[:, t, c * 8 : (c + 1) * 8, 0:HD],
                                    ps[:].rearrange("p (h d) -> p h d", d=HD),
                                )

                    # ---- K^T projection --------------------------------
                    for m in range(DT):
                        wk = wsl.tile([128, DT, 128], f32r, tag="w")
                        nc.sync.dma_start(
                            wk[:],
                            WKT[:, ds(m * 128, 128)].rearrange(
                                "(t p) m -> p t m", p=128
                            ),
                        )
                        for tb in range(S // 512):
                            ps = psA.tile([128, 512], f32, tag="pj")
                            proj_chain(
                                ps,
                                lambda k, m=m: wk[:, k, :],
                                lambda k, tb=tb: x_rhs(tb, k),
                            )
                            nc.vector.tensor_scalar(
                                out=kT[:, m, ds(tb * 512, 512)],
                                in0=ps[:],
                                scalar1=bcols[:, 8 + m : 9 + m],
                                scalar2=None,
                                op0=mybir.AluOpType.add,
                            )

                    # ---- Q^T projection: qb1 head-start ----------------
                    for m in range(2):
                        q_chain(psA, m, 1)
                # xTo closed (32KB/p freed)
            # psA closed (2 banks freed)

            # ---- attention phase ---------------------------------------
            with (
                tc.tile_pool(name="psS", bufs=2, space="PSUM") as psS,
                tc.tile_pool(name="psV", bufs=2, space="PSUM") as psV,
                tc.tile_pool(name="psQ", bufs=2, space="PSUM") as psQ,
                tc.tile_pool(name="wo", bufs=1) as wop,
                tc.tile_pool(name="att", bufs=2) as att,
            ):

                def normalize(pv, attnT, h):
                    hp = (h % 2) * 64
                    rowsum = att.tile([1, 512], f32, tag="rowsum", bufs=1)
                    nc.vector.tensor_copy(rowsum[:], pv[HD : HD + 1, :])
                    recip = att.tile([1, 512], f32, tag="recip", bufs=1)
                    nc.vector.reciprocal_approx_fast(recip[:], rowsum[:])
                    rb = att.tile([64, 512], f32, tag="rb", bufs=1)
                    nc.gpsimd.partition_broadcast(rb[:], recip[:])
                    nc.vector.tensor_mul(
                        attnT[hp : hp + 64, h // 2, :], pv[0:HD, :], rb[:]
                    )

                def pv_mms(pv, e_prev, h, g):
                    for j in range(2):
                        kt = g * 2 + j
                        chain(nc.tensor.matmul(
                            pv[:],
                            vP[:, kt, h, :],
                            e_prev[:, ds(j * 512, 512)],
                            start=(kt == 0),
                            stop=(kt == KT - 1),
                        ))

                # filler generators: real PE work dribbled into the
                # attention stream, one matmul at a time
                def gen_qproj_fillers():
                    # deferred Q-projection chains (qb1, m=2..7)
                    for m in range(2, DT):
                        wq = wsl.tile([128, DT, 128], f32r, tag="w")
                        nc.sync.dma_start(
                            wq[:],
                            WQT[:, ds(m * 128, 128)].rearrange(
                                "(t p) m -> p t m", p=128
                            ),
                        )
                        ps = psQ.tile([128, 512], f32, tag="pq")
                        for k in range(DT):
                            yield lambda k=k, wq=wq, ps=ps: chain(
                                nc.tensor.matmul(
                                    ps[:], wq[:, k, :], x_rhs(1, k),
                                    start=(k == 0), stop=(k == DT - 1),
                                )
                            )
                        def fin(ps=ps, m=m):
                            nc.vector.tensor_scalar(
                                out=qT[:, m, ds(512, 512)],
                                in0=ps[:],
                                scalar1=bcols[:, m : m + 1],
                                scalar2=None,
                                op0=mybir.AluOpType.add,
                            )
                        yield fin

                def gen_outproj_fillers(attnT, qb):
                    for c in range(2):
                        wo = wop.tile([128, DT, 512], f32r, tag="wo")
                        nc.sync.dma_start(
                            wo[:],
                            WOT[:, ds(c * 512, 512)].rearrange(
                                "(t p) m -> p t m", p=128
                            ),
                        )
                        for tt in range(4):
                            ps = psQ.tile([128, 512], f32, tag="pq")
                            for k in range(DT):
                                yield lambda k=k, ps=ps, tt=tt, wo=wo: chain(
                                    nc.tensor.matmul(
                                        ps[:],
                                        attnT[:, k, ds(tt * 128, 128)],
                                        wo[:, k, ds(0, 512)],
                                        start=(k == 0), stop=False,
                                    )
                                )
                            def fin(ps=ps, tt=tt, c=c, qb=qb):
                                chain(nc.tensor.matmul(
                                    ps[:],
                                    onesrow[0:1, 0:128],
                                    biases[0:1, ds(D + c * 512, 512)],
                                    start=False, stop=True,
                                ))
                                y = att.tile([128, 512], f32, tag="y", bufs=1)
                                nc.vector.tensor_copy(y[:], ps[:])
                                nc.sync.dma_start(
                                    OUT[
                                        ds(qb * 512 + tt * 128, 128),
                                        ds(c * 512, 512),
                                    ],
                                    y[:],
                                )
                            yield fin

                def run_attention(qb, attnT, fillers, fill_every, budget=0.0):
                    # flat (head, group) stream; pv lags 2 groups
                    pending = []
                    for h in range(H):
                        hp = (h % 2) * 64
                        m = h // 2
                        pv = psV.tile([HD + 1, 512], f32, tag="pv")
                        for g in range(GPH):
                            sc = psS.tile([128, 1024], f32, tag="sc")
                            for j in range(2):
                                kt = g * 2 + j
                                chain(nc.tensor.matmul(
                                    sc[:, ds(j * 512, 512)],
                                    kT[hp : hp + 64, m, ds(kt * 128, 128)],
                                    qT[hp : hp + 64, m, ds(qb * 512, 512)],
                                    start=True,
                                    stop=True,
                                ))
                            e = epool.tile([128, 1024], bf16, tag="E")
                            nc.scalar.activation(e[:], sc[:], EXP, scale=0.125)
                            pending.append((pv, e, h, g))
                            if len(pending) > 3:
                                p = pending.pop(0)
                                pv_mms(p[0], p[1], p[2], p[3])
                                if p[3] == GPH - 1:
                                    normalize(p[0], attnT, p[2])
                            budget += fill_every
                            while budget >= 1.0:
                                budget -= 1.0
                                try:
                                    next(fillers)()
                                except StopIteration:
                                    budget = -1e9
                    for p in pending:
                        pv_mms(p[0], p[1], p[2], p[3])
                        if p[3] == GPH - 1:
                            normalize(p[0], attnT, p[2])
                    # drain leftover fillers densely
                    for f in fillers:
                        f()

                attnT0 = att.tile([128, DT, 512], f32r, tag="attnT")
                run_attention(0, attnT0, gen_qproj_fillers(), 54 / 128)

                attnT1 = att.tile([128, DT, 512], f32r, tag="attnT")
                run_attention(1, attnT1, gen_outproj_fillers(attnT0, 0), 72 / 124, budget=-4.0)

                # final output projection for qb1 (dense tail)
                for f in gen_outproj_fillers(attnT1, 1):
                    f()


    nc.compile()
    return nc


def _prepare_in_maps(x, Wq, Wk, Wv, Wo, bq, bk, bv, bo):
    import ml_dtypes

    shared = {
        "wqT": _round_fp32r(np.asarray(Wq, np.float32).T),
        "wkT": _round_fp32r(np.asarray(Wk, np.float32).T),
        "wvT": _round_fp32r(np.asarray(Wv, np.float32).T),
        "woT": _round_fp32r(np.asarray(Wo, np.float32).T),
        "biases": _round_fp32r(
            np.concatenate(
                [np.asarray(bv, np.float32), np.asarray(bo, np.float32)]
            )[None, :]
        ),
        "onesrow": np.ones((1, 512), np.float32),
        "biascols": np.concatenate(
            [
                np.asarray(bq, np.float32).reshape(8, 128).T,
                np.asarray(bk, np.float32).reshape(8, 128).T,
            ],
            axis=1,
        ),
        "onescol": np.ones((128, 1), ml_dtypes.bfloat16),
    }
    x = np.asarray(x, np.float32)
    in_maps = []
    for c in range(N_CORES):
        b, half = divmod(c, 2)
        own = x[b, half * TQ : (half + 1) * TQ]
        other = x[b, (1 - half) * TQ : (2 - half) * TQ]
        in_maps.append(
            {
                **shared,
                "xTq": _round_fp32r(np.ascontiguousarray(own.T)),
                "xTo": _round_fp32r(np.ascontiguousarray(other.T)),
            }
        )
    return in_maps


def kernel(x, Wq, Wk, Wv, Wo, bq, bk, bv, bo):
    from concourse.bass_utils import run_bass_kernel_spmd

    if "nc" not in _CACHE:
        _CACHE["nc"] = _build_nc()
    nc = _CACHE["nc"]

    in_maps = _prepare_in_maps(x, Wq, Wk, Wv, Wo, bq, bk, bv, bo)
    res = run_bass_kernel_spmd(nc, in_maps, list(range(N_CORES)))

    out = np.empty((B, S, D), np.float32)
    for c in range(N_CORES):
        b, half = divmod(c, 2)
        out[b, half * TQ : (half + 1) * TQ] = res.results[c]["out"]
    return out


# revision 40
# speedup vs baseline: 1.0323x; 1.0323x over previous
"""Multi-head attention (B=4, S=2048, D=1024, H=16) on 8 TRN2 NeuronCores.

Sharding: token-parallel, zero-collective. Core c handles batch c//2 and
half (c%2) of that batch's sequence as queries (1024 tokens); it computes
K/V for the batch's full 2048 tokens locally (work duplicated x2 across the
pair, but no cross-core communication at all).

The host reorders tokens per-core so every core's own query tokens are
always columns [0, 1024) of its xT input -> all 8 cores run the identical
SPMD graph. Key order within a batch differs per core, but attention is
permutation-invariant over keys.

On-device layout is "transposed" (channels x tokens) so no transposes:
  Q^T = WqT.T @ xT      (bias via K=1 ones-row matmul into PSUM)
  S^T = K^T_head.T @ Q^T_head   (contraction over head_dim=64)
  E   = exp(S^T / 8)    (ScalarE, reads PSUM, 2 key-tiles per ACTIVATE)
  PV  = V'_head.T @ E   (V' has a ones column -> row 64 = softmax denom)
  out = attnT.T @ WoT   (natural token-major output, contiguous DMA)

The attention phase is choreographed for the PE clock-gate (HAM): the PE
must never stall on a semaphore or it drops to 1.2 GHz. Scores(i) ->
exp(i) -> pv(i-2) software pipelining plus real filler work (the deferred
qb1 Q-projection during qb0, and qb0's output projection during qb1) keep
the TensorE queue saturated while ScalarE streams exps back-to-back; an
explicit add_dep chain pins the PE order so the tile scheduler cannot
re-serialize it.

Precision: projections + output in fp32r (fp32 with 11-bit mantissa, full
TensorE rate, host pre-rounds inputs); attention QK/PV in bf16;
accumulation always fp32 in PSUM. Softmax denominators via DVE
reciprocal_approx_fast + gpsimd partition_broadcast.
"""

import numpy as np

D = 1024
H = 16
HD = 64
B = 4
S = 2048
TQ = 1024  # query tokens per core
N_CORES = 8

_CACHE: dict = {}


def _round_fp32r(x: np.ndarray) -> np.ndarray:
    """Round-to-nearest-even fp32 mantissa to 11 bits (fp32r)."""
    b = np.ascontiguousarray(x, dtype=np.float32).view(np.uint32).astype(np.uint64)
    out = ((b + 0x7FF + ((b >> 12) & 1)) & 0xFFFFF000).astype(np.uint32)
    return out.view(np.float32).reshape(x.shape)


def _build_nc():
    import concourse.tile as tile
    import concourse.mybir as mybir
    from concourse import bacc
    from concourse.bass import ds
    from concourse.tile import add_dep_helper

    f32 = mybir.dt.float32
    f32r = mybir.dt.float32r
    bf16 = mybir.dt.bfloat16
    EXP = mybir.ActivationFunctionType.Exp

    nc = bacc.Bacc()

    XTQ = nc.declare_dram_parameter("xTq", [D, TQ], f32r, isOutput=False)
    XTO = nc.declare_dram_parameter("xTo", [D, TQ], f32r, isOutput=False)
    WQT = nc.declare_dram_parameter("wqT", [D, D], f32r, isOutput=False)
    WKT = nc.declare_dram_parameter("wkT", [D, D], f32r, isOutput=False)
    WVT = nc.declare_dram_parameter("wvT", [D, D], f32r, isOutput=False)
    WOT = nc.declare_dram_parameter("woT", [D, D], f32r, isOutput=False)
    BIAS = nc.declare_dram_parameter("biases", [1, 2 * D], f32r, isOutput=False)
    ONESR = nc.declare_dram_parameter("onesrow", [1, 512], f32r, isOutput=False)
    BCOLS = nc.declare_dram_parameter("biascols", [128, 16], f32, isOutput=False)
    ONESC = nc.declare_dram_parameter("onescol", [128, 1], bf16, isOutput=False)
    OUT = nc.declare_dram_parameter("out", [TQ, D], f32, isOutput=True)

    KT = S // 128   # 16 key tiles
    DT = D // 128   # 8 channel tiles
    GPH = KT // 2   # 8 score groups (2 key-tiles each) per head

    pe_chain = [None]

    def chain(bi):
        if pe_chain[0] is not None:
            add_dep_helper(
                bi.ins, pe_chain[0].ins, sync=False, reason="PE order chain"
            )
        pe_chain[0] = bi
        return bi

    with tile.TileContext(nc) as tc:
        with (
            tc.tile_pool(name="pers", bufs=1) as pers,
            tc.tile_pool(name="epool", bufs=3) as epool,
            tc.tile_pool(name="xtq", bufs=1) as xtq,
            tc.tile_pool(name="wsl", bufs=3) as wsl,
        ):
            # ---- persistent tiles --------------------------------------
            kT = pers.tile([128, DT, S], bf16)          # K^T  (ch x tok)
            qT = pers.tile([128, DT, TQ], bf16)         # Q^T  (ch x tok)
            vP = pers.tile([128, KT, H, HD + 1], bf16)  # V'  (+ones col)
            biases = pers.tile([1, 2 * D], f32r)
            onesrow = pers.tile([1, 512], f32r)
            onescol = pers.tile([128, 1], bf16)
            bcols = pers.tile([128, 16], f32)
            scratch = pers.tile([1, 16], f32)

            # setup + bulk prefetches ride the ScalarE HWDGE queue so the
            # sync queue carries only the critical path (xTq h1, wq slices)
            nc.scalar.dma_start(bcols[:], BCOLS[:])
            nc.scalar.dma_start(biases[:], BIAS[:])
            nc.scalar.dma_start(onesrow[:], ONESR[:])
            nc.scalar.dma_start(onescol[:], ONESC[:])

            # warm the exp table early (one-time ~2.7us ACT table load)
            nc.vector.memset(scratch[:], 0.0)
            nc.scalar.activation(scratch[0:1, 0:16], scratch[0:1, 0:16], EXP)

            # V' ones columns (softmax denominator trick)
            for t in range(KT):
                nc.vector.tensor_copy(
                    vP[:, t, :, HD], onescol[:, 0:1].broadcast_to([128, H])
                )

            xTq = xtq.tile([128, DT, TQ], f32r)  # own-token half of x^T
            nc.sync.dma_start(
                xTq[:, ds(0, 4), :],
                XTQ[ds(0, 512), :].rearrange("(t p) s -> p t s", p=128),
            )
            nc.scalar.dma_start(
                xTq[:, ds(4, 4), :],
                XTQ[ds(512, 512), :].rearrange("(t p) s -> p t s", p=128),
            )

            def x_lhsT(t, k):
                # token-tile t (128 tokens), channel-tile k -> [128, 128]
                if t < 8:
                    return xTq[:, k, ds(t * 128, 128)]
                return xTo[:, k, ds((t - 8) * 128, 128)]

            def x_rhs(tb, k):
                # 512-token block tb, channel-tile k -> [128, 512]
                if tb < 2:
                    return xTq[:, k, ds(tb * 512, 512)]
                return xTo[:, k, ds((tb - 2) * 512, 512)]

            def proj_chain(ps, lhsT_fn, rhs_fn, bias_lhsT=None, bias_rhs=None):
                has_bias = bias_lhsT is not None
                for k in range(DT):
                    chain(nc.tensor.matmul(
                        ps[:], lhsT_fn(k), rhs_fn(k),
                        start=(k == 0), stop=(not has_bias and k == DT - 1),
                    ))
                if has_bias:
                    chain(nc.tensor.matmul(
                        ps[:], bias_lhsT, bias_rhs, start=False, stop=True,
                    ))

            def q_chain(psQ, m, tb):
                wq = wsl.tile([128, DT, 128], f32r, tag="w")
                nc.sync.dma_start(
                    wq[:],
                    WQT[:, ds(m * 128, 128)].rearrange("(t p) m -> p t m", p=128),
                )
                ps = psQ.tile([128, 512], f32, tag="pq")
                for k in range(DT):
                    chain(nc.tensor.matmul(
                        ps[:], wq[:, k, :], x_rhs(tb, k),
                        start=(k == 0), stop=(k == DT - 1),
                    ))
                nc.vector.tensor_scalar(
                    out=qT[:, m, ds(tb * 512, 512)],
                    in0=ps[:],
                    scalar1=bcols[:, m : m + 1],
                    scalar2=None,
                    op0=mybir.AluOpType.add,
                )

            with tc.tile_pool(name="psA", bufs=2, space="PSUM") as psA:
                with tc.tile_pool(name="xto", bufs=1) as xto:
                    with tc.tile_pool(name="wv", bufs=2) as wvp:
                        # prefetch: wv chunk 0 + other-half x^T land while
                        # the PE runs the Q-projection below
                        wv_tiles = []
                        wv_tiles.append(wvp.tile([128, DT, 512], f32r, tag="wv", name="wv0"))
                        nc.scalar.dma_start(
                            wv_tiles[0][:],
                            WVT[:, ds(0, 512)].rearrange("(t p) m -> p t m", p=128),
                        )
                        xTo = xto.tile([128, DT, TQ], f32r)  # other-token half
                        nc.scalar.dma_start(
                            xTo[:], XTO[:].rearrange("(t p) s -> p t s", p=128)
                        )
                        wv_tiles.append(wvp.tile([128, DT, 512], f32r, tag="wv", name="wv1"))
                        nc.scalar.dma_start(
                            wv_tiles[1][:],
                            WVT[:, ds(512, 512)].rearrange("(t p) m -> p t m", p=128),
                        )

                        # ---- Q^T projection qb0 first: only needs xTq ----
                        for m in range(DT):
                            q_chain(psA, m, 0)

                        # ---- V projection (wvT in two halves) ------------
                        for c in range(2):
                            wv = wv_tiles[c]
                            for t in range(KT):
                                ps = psA.tile([128, 512], f32, tag="pj")
                                proj_chain(
                                    ps,
                                    lambda k, t=t: x_lhsT(t, k),
                                    lambda k, wv=wv: wv[:, k, :],
                                    onesrow[0:1, 0:128],
                                    biases[0:1, ds(c * 512, 512)],
                                )
                                nc.vector.tensor_copy(
                                    vP[:, t, c * 8 : (c + 1) * 8, 0:HD],
                                    ps[:].rearrange("p (h d) -> p h d", d=HD),
                                )

                    # ---- K^T projection --------------------------------
                    for m in range(DT):
                        wk = wsl.tile([128, DT, 128], f32r, tag="w")
                        nc.sync.dma_start(
                            wk[:],
                            WKT[:, ds(m * 128, 128)].rearrange(
                                "(t p) m -> p t m", p=128
                            ),
                        )
                        for tb in range(S // 512):
                            ps = psA.tile([128, 512], f32, tag="pj")
                            proj_chain(
                                ps,
                                lambda k, m=m: wk[:, k, :],
                                lambda k, tb=tb: x_rhs(tb, k),
                            )
                            nc.vector.tensor_scalar(
                                out=kT[:, m, ds(tb * 512, 512)],
                                in0=ps[:],
                                scalar1=bcols[:, 8 + m : 9 + m],
                                scalar2=None,
                                op0=mybir.AluOpType.add,
                            )

                    # ---- Q^T projection: qb1 head-start ----------------
                    for m in range(2):
                        q_chain(psA, m, 1)
                # xTo closed (32KB/p freed)
            # psA closed (2 banks freed)

            # ---- attention phase ---------------------------------------
            with (
                tc.tile_pool(name="psS", bufs=2, space="PSUM") as psS,
                tc.tile_pool(name="psV", bufs=2, space="PSUM") as psV,
                tc.tile_pool(name="psQ", bufs=2, space="PSUM") as psQ,
                tc.tile_pool(name="wo", bufs=1) as wop,
                tc.tile_pool(name="att", bufs=2) as att,
            ):

                def normalize(pv, attnT, h):
                    hp = (h % 2) * 64
                    rowsum = att.tile([1, 512], f32, tag="rowsum", bufs=1)
                    nc.vector.tensor_copy(rowsum[:], pv[HD : HD + 1, :])
                    recip = att.tile([1, 512], f32, tag="recip", bufs=1)
                    nc.vector.reciprocal_approx_fast(recip[:], rowsum[:])
                    rb = att.tile([64, 512], f32, tag="rb", bufs=1)
                    nc.gpsimd.partition_broadcast(rb[:], recip[:])
                    nc.vector.tensor_mul(
                        attnT[hp : hp + 64, h // 2, :], pv[0:HD, :], rb[:]
                    )

                def pv_mms(pv, e_prev, h, g):
                    for j in range(2):
                        kt = g * 2 + j
                        chain(nc.tensor.matmul(
                            pv[:],
                            vP[:, kt, h, :],
                            e_prev[:, ds(j * 512, 512)],
                            start=(kt == 0),
                            stop=(kt == KT - 1),
                        ))

                # filler generators: real PE work dribbled into the
                # attention stream, one matmul at a time
                def gen_qproj_fillers():
                    # deferred Q-projection chains (qb1, m=2..7)
                    for m in range(2, DT):
                        wq = wsl.tile([128, DT, 128], f32r, tag="w")
                        nc.sync.dma_start(
                            wq[:],
                            WQT[:, ds(m * 128, 128)].rearrange(
                                "(t p) m -> p t m", p=128
                            ),
                        )
                        ps = psQ.tile([128, 512], f32, tag="pq")
                        for k in range(DT):
                            yield lambda k=k, wq=wq, ps=ps: chain(
                                nc.tensor.matmul(
                                    ps[:], wq[:, k, :], x_rhs(1, k),
                                    start=(k == 0), stop=(k == DT - 1),
                                )
                            )
                        def fin(ps=ps, m=m):
                            nc.vector.tensor_scalar(
                                out=qT[:, m, ds(512, 512)],
                                in0=ps[:],
                                scalar1=bcols[:, m : m + 1],
                                scalar2=None,
                                op0=mybir.AluOpType.add,
                            )
                        yield fin

                def gen_outproj_fillers(attnT, qb):
                    for c in range(2):
                        wo = wop.tile([128, DT, 512], f32r, tag="wo")
                        nc.sync.dma_start(
                            wo[:],
                            WOT[:, ds(c * 512, 512)].rearrange(
                                "(t p) m -> p t m", p=128
                            ),
                        )
                        for tt in range(4):
                            ps = psQ.tile([128, 512], f32, tag="pq")
                            for k in range(DT):
                                yield lambda k=k, ps=ps, tt=tt, wo=wo: chain(
                                    nc.tensor.matmul(
                                        ps[:],
                                        attnT[:, k, ds(tt * 128, 128)],
                                        wo[:, k, ds(0, 512)],
                                        start=(k == 0), stop=False,
                                    )
                                )
                            def fin(ps=ps, tt=tt, c=c, qb=qb):
                                chain(nc.tensor.matmul(
                                    ps[:],
                                    onesrow[0:1, 0:128],
                                    biases[0:1, ds(D + c * 512, 512)],
                                    start=False, stop=True,
                                ))
                                y = att.tile([128, 512], f32, tag="y", bufs=1)
                                nc.vector.tensor_copy(y[:], ps[:])
                                nc.sync.dma_start(
                                    OUT[
                                        ds(qb * 512 + tt * 128, 128),
                                        ds(c * 512, 512),
                                    ],
                                    y[:],
                                )
                            yield fin

                def run_attention(qb, attnT, fillers, fill_every, budget=0.0):
                    # flat (head, group) stream; pv lags 2 groups
                    pending = []
                    for h in range(H):
                        hp = (h % 2) * 64
                        m = h // 2
                        pv = psV.tile([HD + 1, 512], f32, tag="pv")
                        for g in range(GPH):
                            sc = psS.tile([128, 1024], f32, tag="sc")
                            for j in range(2):
                                kt = g * 2 + j
                                chain(nc.tensor.matmul(
                                    sc[:, ds(j * 512, 512)],
                                    kT[hp : hp + 64, m, ds(kt * 128, 128)],
                                    qT[hp : hp + 64, m, ds(qb * 512, 512)],
                                    start=True,
                                    stop=True,
                                ))
                            e = epool.tile([128, 1024], bf16, tag="E")
                            nc.scalar.activation(e[:], sc[:], EXP, scale=0.125)
                            pending.append((pv, e, h, g))
                            if len(pending) > 2:
                                p = pending.pop(0)
                                pv_mms(p[0], p[1], p[2], p[3])
                                if p[3] == GPH - 1:
                                    normalize(p[0], attnT, p[2])
                            budget += fill_every
                            while budget >= 1.0:
                                budget -= 1.0
                                try:
                                    next(fillers)()
                                except StopIteration:
                                    budget = -1e9
                    for p in pending:
                        pv_mms(p[0], p[1], p[2], p[3])
                        if p[3] == GPH - 1:
                            normalize(p[0], attnT, p[2])
                    # drain leftover fillers densely
                    for f in fillers:
                        f()

                attnT0 = att.tile([128, DT, 512], f32r, tag="attnT")
                run_attention(0, attnT0, gen_qproj_fillers(), 54 / 128)

                attnT1 = att.tile([128, DT, 512], f32r, tag="attnT")
                run_attention(1, attnT1, gen_outproj_fillers(attnT0, 0), 72 / 124, budget=-4.0)

                # final output projection for qb1 (dense tail)
                for f in gen_outproj_fillers(attnT1, 1):
                    f()


    nc.compile()
    return nc


def _prepare_in_maps(x, Wq, Wk, Wv, Wo, bq, bk, bv, bo):
    import ml_dtypes

    shared = {
        "wqT": _round_fp32r(np.asarray(Wq, np.float32).T),
        "wkT": _round_fp32r(np.asarray(Wk, np.float32).T),
        "wvT": _round_fp32r(np.asarray(Wv, np.float32).T),
        "woT": _round_fp32r(np.asarray(Wo, np.float32).T),
        "biases": _round_fp32r(
            np.concatenate(
                [np.asarray(bv, np.float32), np.asarray(bo, np.float32)]
            )[None, :]
        ),
        "onesrow": np.ones((1, 512), np.float32),
        "biascols": np.concatenate(
            [
                np.asarray(bq, np.float32).reshape(8, 128).T,
                np.asarray(bk, np.float32).reshape(8, 128).T,
            ],
            axis=1,
        ),
        "onescol": np.ones((128, 1), ml_dtypes.bfloat16),
    }
    x = np.asarray(x, np.float32)
    in_maps = []
    for c in range(N_CORES):
        b, half = divmod(c, 2)
        own = x[b, half * TQ : (half + 1) * TQ]
        other = x[b, (1 - half) * TQ : (2 - half) * TQ]
        in_maps.append(
            {
                **shared,
                "xTq": _round_fp32r(np.ascontiguousarray(own.T)),
                "xTo": _round_fp32r(np.ascontiguousarray(other.T)),
            }
        )
    return in_maps


def kernel(x, Wq, Wk, Wv, Wo, bq, bk, bv, bo):
    from concourse.bass_utils import run_bass_kernel_spmd

    if "nc" not in _CACHE:
        _CACHE["nc"] = _build_nc()
    nc = _CACHE["nc"]

    in_maps = _prepare_in_maps(x, Wq, Wk, Wv, Wo, bq, bk, bv, bo)
    res = run_bass_kernel_spmd(nc, in_maps, list(range(N_CORES)))

    out = np.empty((B, S, D), np.float32)
    for c in range(N_CORES):
        b, half = divmod(c, 2)
        out[b, half * TQ : (half + 1) * TQ] = res.results[c]["out"]
    return out
